# revision 1
# baseline (speedup 1.0000x reference)
"""GravityAE GNN message-passing kernel for 8 TRN2 NeuronCores (Bass/Tile).

Algorithm (see reference GCN autoencoder):
  scale_k = gamma_k / sqrt(var_k + eps); shift_k = beta_k + (b_k - mean_k)*scale_k
  W1p = W1 * scale1; W2p = W2 * scale2
  dinv[n] = 1/sqrt(in_degree incl self loop)
  xs1 = dinv * (x @ W1p)                     (node table, gathered by src)
  h   = leaky(dinv[d] * segsum_d(xs1[src]) + shift1)
  hw2 = dinv * (h @ W2p)
  z   = leaky(dinv[d] * segsum_d(hw2[src]) + shift2)
  out[e] = sigmoid(z[dst,-1] - ||z[src,:-1] - z[dst,:-1]||)

Distribution: aggregation is dst-sharded (each core owns a contiguous range
of 128-node windows; edges sorted by dst, bucketed per window, padded to a
uniform C_max chunks of 128 edges). Per chunk the segment-sum is an
indicator-matrix matmul accumulated in PSUM:  S[e,m] = (dst_local[e]==m),
PSUM += S^T @ gathered_rows.  AllGather (x8) rebuilds the full node tables
between stages.  Decode is edge-sharded in original order.
"""
import math
import numpy as np

P = 128
EPS = 1e-5


# --------------------------------------------------------------------------
# host-side preprocessing
# --------------------------------------------------------------------------
def _build_host_tables(x, edge_index, n_cores):
    N = x.shape[0]
    E = edge_index.shape[1]
    NW = ((N + P - 1) // P + n_cores - 1) // n_cores * n_cores  # windows, mult of n_cores
    NP = NW * P
    src = edge_index[0].astype(np.int64)
    dst = edge_index[1].astype(np.int64)
    s_all = np.concatenate([src, np.arange(N)])
    d_all = np.concatenate([dst, np.arange(N)])
    deg = np.bincount(d_all, minlength=NP).astype(np.float64)
    dinv = np.zeros(NP, np.float32)
    nz = deg > 0
    dinv[nz] = (1.0 / np.sqrt(deg[nz])).astype(np.float32)

    order = np.argsort(d_all, kind="stable")
    s_sorted = d_sorted = None
    s_sorted = s_all[order]
    d_sorted = d_all[order]
    win_of_edge = d_sorted // P
    counts = np.bincount(win_of_edge, minlength=NW)
    C_max = max(1, int(np.ceil(counts.max() / P)))
    CW = C_max * P

    offs = np.full((NW, CW), NP - 1, np.int32)   # pad slots -> last (all-zero) row
    dstf = np.full((NW, CW), -1.0, np.float32)   # pad slots -> never match iota
    starts = np.zeros(NW + 1, np.int64)
    np.cumsum(counts, out=starts[1:])
    for w in range(NW):
        c = counts[w]
        offs[w, :c] = s_sorted[starts[w] : starts[w] + c]
        dstf[w, :c] = (d_sorted[starts[w] : starts[w] + c] - w * P).astype(np.float32)

    # per-core slot tables, SBUF layout [P, NWc*C_max]
    NWc = NW // n_cores
    offs_core = np.empty((n_cores, P, NWc * C_max), np.int32)
    dstf_core = np.empty((n_cores, P, NWc * C_max), np.float32)
    for c in range(n_cores):
        blk_o = offs[c * NWc : (c + 1) * NWc].reshape(NWc, C_max, P)   # [wl, cc, p]
        blk_d = dstf[c * NWc : (c + 1) * NWc].reshape(NWc, C_max, P)
        offs_core[c] = blk_o.transpose(2, 0, 1).reshape(P, NWc * C_max)
        dstf_core[c] = blk_d.transpose(2, 0, 1).reshape(P, NWc * C_max)

    # decode tables: edges in original order, sharded contiguously
    EC = (E + n_cores - 1) // n_cores          # real edges per core (last short)
    DG = (EC + 2047) // 2048                   # groups of 2048 edges
    didx_core = np.zeros((n_cores, P, DG * 32), np.int32)
    for c in range(n_cores):
        e0, e1 = c * EC, min((c + 1) * EC, E)
        n = e1 - e0
        sp = np.zeros(DG * 2048, np.int64)
        dp = np.zeros(DG * 2048, np.int64)
        sp[:n] = src[e0:e1]
        dp[:n] = dst[e0:e1]
        sp3 = sp.reshape(DG, 16, P)            # [g, cc, p]
        dp3 = dp.reshape(DG, 16, P)
        blk = np.concatenate([sp3, dp3], axis=1)      # [g, 32, p]
        didx_core[c] = blk.transpose(2, 0, 1).reshape(P, DG * 32)

    return dict(N=N, E=E, NW=NW, NP=NP, C_max=C_max, NWc=NWc, EC=EC, DG=DG,
                dinv=dinv, offs_core=offs_core, dstf_core=dstf_core,
                didx_core=didx_core)


# --------------------------------------------------------------------------
# bass program
# --------------------------------------------------------------------------
def _build_program(NP, NWc, C_max, F1, F2, DG, n_cores, big_iseq=True):
    import concourse.bass as bass
    import concourse.tile as tile
    from concourse import bacc, mybir

    dt = mybir.dt
    f32 = dt.float32
    i32 = dt.int32
    Nc = NWc * P
    CW = C_max * P
    OB = (DG * 16 + P - 1) // P                 # output transpose blocks
    OUTLEN = OB * P * P

    nc = bacc.Bacc("TRN2", target_bir_lowering=False, debug=False,
                   num_devices=n_cores)
    x_in = nc.declare_dram_parameter("x", [Nc, F1], f32, isOutput=False)
    w1_in = nc.declare_dram_parameter("w1", [F1, F1], f32, isOutput=False)
    w2_in = nc.declare_dram_parameter("w2", [F1, F2], f32, isOutput=False)
    sh1_in = nc.declare_dram_parameter("shift1", [P, F1], f32, isOutput=False)
    sh2_in = nc.declare_dram_parameter("shift2", [P, F2], f32, isOutput=False)
    iota_in = nc.declare_dram_parameter("iota", [P, CW], f32, isOutput=False)
    id_in = nc.declare_dram_parameter("ident", [P, P], f32, isOutput=False)
    dinv_in = nc.declare_dram_parameter("dinv", [Nc, 1], f32, isOutput=False)
    offs_in = nc.declare_dram_parameter("offs", [P, NWc * C_max], i32, isOutput=False)
    dstf_in = nc.declare_dram_parameter("dstf", [P, NWc * C_max], f32, isOutput=False)
    didx_in = nc.declare_dram_parameter("didx", [P, DG * 32], i32, isOutput=False)
    out_dram = nc.declare_dram_parameter("out", [OUTLEN], f32, isOutput=True)

    rg = [list(range(n_cores))]

    with tile.TileContext(nc) as tc:
        with (
            tc.tile_pool(name="const", bufs=1) as cpool,
            tc.tile_pool(name="sbuf", bufs=3) as pool,
            tc.tile_pool(name="psA", bufs=2, space="PSUM") as psA,
            tc.tile_pool(name="dram", bufs=1, space="DRAM") as dpool,
        ):
            # ---- constants ----
            w1_t = cpool.tile([F1, F1], f32)
            w2_t = cpool.tile([F1, F2], f32)
            sh1_t = cpool.tile([P, F1], f32)
            sh2_t = cpool.tile([P, F2], f32)
            iota_t = cpool.tile([P, CW], f32)
            id_t = cpool.tile([P, P], f32)
            nc.sync.dma_start(out=w1_t[:], in_=w1_in[:])
            nc.sync.dma_start(out=w2_t[:], in_=w2_in[:])
            nc.sync.dma_start(out=sh1_t[:], in_=sh1_in[:])
            nc.sync.dma_start(out=sh2_t[:], in_=sh2_in[:])
            nc.sync.dma_start(out=iota_t[:], in_=iota_in[:])
            nc.sync.dma_start(out=id_t[:], in_=id_in[:])

            # ---- collective buffers ----
            ag1_in = dpool.tile([Nc, F1], f32)
            xs1_full = dpool.tile([NP, F1], f32, addr_space="Shared")
            ag2_in = dpool.tile([Nc, F2], f32)
            hw2_full = dpool.tile([NP, F2], f32, addr_space="Shared")
            ag3_in = dpool.tile([Nc, F2], f32)
            z_full = dpool.tile([NP, F2], f32, addr_space="Shared")

            # ---- stage A: xs1 shard = dinv * (x @ W1p) ----
            for w in range(NWc):
                x_t = pool.tile([P, F1], f32, tag="ax")
                dv_t = pool.tile([P, 1], f32, tag="adv")
                nc.sync.dma_start(out=x_t[:], in_=x_in[w * P:(w + 1) * P, :])
                nc.sync.dma_start(out=dv_t[:], in_=dinv_in[w * P:(w + 1) * P, :])
                ps_x = psA.tile([P, P], f32, tag="xp")
                nc.tensor.transpose(ps_x[:], x_t[:], id_t[:])
                xT_t = pool.tile([P, P], f32, tag="axT")
                nc.vector.tensor_copy(xT_t[:], ps_x[:])
                ps_mm = psA.tile([P, F1], f32, tag="mm")
                nc.tensor.matmul(ps_mm[:], xT_t[:], w1_t[:], start=True, stop=True)
                xs_t = pool.tile([P, F1], f32, tag="axs")
                nc.vector.tensor_scalar_mul(xs_t[:], ps_mm[:], dv_t[:, :1])
                nc.sync.dma_start(out=ag1_in[w * P:(w + 1) * P, :], in_=xs_t[:])

            nc.gpsimd.collective_compute(
                "AllGather", mybir.AluOpType.bypass,
                ins=[ag1_in.opt()], outs=[xs1_full.opt()], replica_groups=rg)

            # ---- aggregation layer (shared code for L1 / L2) ----
            def agg_layer(table_full, F, w_t, sh_t, store_cb, tagp):
                for w in range(NWc):
                    of_t = pool.tile([P, C_max], i32, tag=tagp + "of")
                    df_t = pool.tile([P, C_max], f32, tag=tagp + "df")
                    dv_t = pool.tile([P, 1], f32, tag=tagp + "dv")
                    nc.sync.dma_start(out=of_t[:], in_=offs_in[:, w * C_max:(w + 1) * C_max])
                    nc.sync.dma_start(out=df_t[:], in_=dstf_in[:, w * C_max:(w + 1) * C_max])
                    nc.sync.dma_start(out=dv_t[:], in_=dinv_in[w * P:(w + 1) * P, :])
                    msg_t = pool.tile([P, C_max, F], f32, tag=tagp + "msg")
                    for c in range(C_max):
                        nc.gpsimd.indirect_dma_start(
                            out=msg_t[:, c, :], out_offset=None,
                            in_=table_full[:],
                            in_offset=bass.IndirectOffsetOnAxis(ap=of_t[:, c:c + 1], axis=0))
                    S_t = pool.tile([P, C_max, P], f32, tag=tagp + "S")
                    if big_iseq:
                        nc.vector.tensor_tensor(
                            out=S_t[:],
                            in0=df_t[:].rearrange("p (c o) -> p c o", o=1).to_broadcast([P, C_max, P]),
                            in1=iota_t[:].rearrange("p (c m) -> p c m", m=P),
                            op=mybir.AluOpType.is_equal)
                    else:
                        for c in range(C_max):
                            nc.vector.tensor_tensor(
                                out=S_t[:, c, :],
                                in0=df_t[:, c:c + 1].to_broadcast([P, P]),
                                in1=iota_t[:, :P],
                                op=mybir.AluOpType.is_equal)
                    ps_agg = psA.tile([P, F], f32, tag="agg")
                    for c in range(C_max):
                        nc.tensor.matmul(ps_agg[:], S_t[:, c, :], msg_t[:, c, :],
                                         start=(c == 0), stop=(c == C_max - 1))
                    t1 = pool.tile([P, F], f32, tag=tagp + "t1")
                    nc.vector.tensor_scalar_mul(t1[:], ps_agg[:], dv_t[:, :1])
                    t2 = pool.tile([P, F], f32, tag=tagp + "t2")
                    nc.vector.tensor_tensor(out=t2[:], in0=t1[:], in1=sh_t[:],
                                            op=mybir.AluOpType.add)
                    u_t = pool.tile([P, F], f32, tag=tagp + "u")
                    nc.scalar.activation(u_t[:], t2[:],
                                         mybir.ActivationFunctionType.Copy,
                                         scale=0.1)
                    o_t = pool.tile([P, F], f32, tag=tagp + "o")
                    nc.vector.tensor_tensor(out=o_t[:], in0=t2[:], in1=u_t[:],
                                            op=mybir.AluOpType.max)
                    store_cb(w, o_t, dv_t)

            # ---- stage B: h windows + hw2 shard ----
            def store_h(w, h_t, dv_t):
                ps_hx = psA.tile([P, P], f32, tag="xp")
                nc.tensor.transpose(ps_hx[:], h_t[:], id_t[:])
                hT_t = pool.tile([P, P], f32, tag="bhT")
                nc.vector.tensor_copy(hT_t[:], ps_hx[:])
                ps_m2 = psA.tile([P, F2], f32, tag="mm")
                nc.tensor.matmul(ps_m2[:], hT_t[:], w2_t[:], start=True, stop=True)
                hw_t = pool.tile([P, F2], f32, tag="bhw")
                nc.vector.tensor_scalar_mul(hw_t[:], ps_m2[:], dv_t[:, :1])
                nc.sync.dma_start(out=ag2_in[w * P:(w + 1) * P, :], in_=hw_t[:])

            agg_layer(xs1_full, F1, w1_t, sh1_t, store_h, "b")

            nc.gpsimd.collective_compute(
                "AllGather", mybir.AluOpType.bypass,
                ins=[ag2_in.opt()], outs=[hw2_full.opt()], replica_groups=rg)

            # ---- stage C: z shard ----
            def store_z(w, z_t, dv_t):
                nc.sync.dma_start(out=ag3_in[w * P:(w + 1) * P, :], in_=z_t[:])

            agg_layer(hw2_full, F2, w2_t, sh2_t, store_z, "c")

            nc.gpsimd.collective_compute(
                "AllGather", mybir.AluOpType.bypass,
                ins=[ag3_in.opt()], outs=[z_full.opt()], replica_groups=rg)

            # ---- decode ----
            Fp = F2 - 1  # position dims
            stage_ss = cpool.tile([P, OB * P], f32)
            stage_mj = cpool.tile([P, OB * P], f32)
            nc.vector.memset(stage_ss[:], 0.0)
            nc.vector.memset(stage_mj[:], 0.0)
            for g in range(DG):
                di_t = pool.tile([P, 32], i32, tag="ddi")
                nc.sync.dma_start(out=di_t[:], in_=didx_in[:, g * 32:(g + 1) * 32])
                zz_t = pool.tile([P, 32, F2], f32, tag="dzz")
                for c in range(32):
                    nc.gpsimd.indirect_dma_start(
                        out=zz_t[:, c, :], out_offset=None,
                        in_=z_full[:],
                        in_offset=bass.IndirectOffsetOnAxis(ap=di_t[:, c:c + 1], axis=0))
                df_t = pool.tile([P, 16, Fp], f32, tag="ddf")
                nc.vector.tensor_tensor(out=df_t[:], in0=zz_t[:, 0:16, 0:Fp],
                                        in1=zz_t[:, 16:32, 0:Fp],
                                        op=mybir.AluOpType.subtract)
                sq_t = pool.tile([P, 16, Fp], f32, tag="dsq")
                nc.scalar.square(sq_t[:], df_t[:])
                nc.vector.reduce_sum(
                    out=stage_ss[:, g * 16:(g + 1) * 16].rearrange("p (c o) -> p c o", o=1),
                    in_=sq_t[:], axis=mybir.AxisListType.X)
                nc.vector.tensor_copy(stage_mj[:, g * 16:(g + 1) * 16],
                                   zz_t[:, 16:32, Fp])
            # finale: sigmoid(mj - sqrt(ss)), transpose, store
            st_d = cpool.tile([P, OB * P], f32)
            nc.scalar.sqrt(st_d[:], stage_ss[:])
            st_v = cpool.tile([P, OB * P], f32)
            nc.vector.tensor_tensor(out=st_v[:], in0=stage_mj[:], in1=st_d[:],
                                    op=mybir.AluOpType.subtract)
            st_o = cpool.tile([P, OB * P], f32)
            nc.scalar.activation(st_o[:], st_v[:],
                                 mybir.ActivationFunctionType.Sigmoid)
            for b in range(OB):
                ps_t = psA.tile([P, P], f32, tag="xp")
                nc.tensor.transpose(ps_t[:], st_o[:, b * P:(b + 1) * P], id_t[:])
                ob_t = pool.tile([P, P], f32, tag="dob")
                nc.vector.tensor_copy(ob_t[:], ps_t[:])
                nc.sync.dma_start(
                    out=out_dram[b * P * P:(b + 1) * P * P].rearrange("(a b) -> a b", b=P),
                    in_=ob_t[:])
    nc.compile()
    return nc


# --------------------------------------------------------------------------
# public entry
# --------------------------------------------------------------------------
def _prep_inputs(x, edge_index, W1, b1, gamma1, beta1, mean1, var1,
                 W2, b2, gamma2, beta2, mean2, var2, n_cores):
    x = np.asarray(x, np.float32)
    edge_index = np.asarray(edge_index)
    ht = _build_host_tables(x, edge_index, n_cores)
    NP, NWc, C_max, DG = ht["NP"], ht["NWc"], ht["C_max"], ht["DG"]
    F1 = W1.shape[1]
    F2 = W2.shape[1]
    Nc = NWc * P
    CW = C_max * P

    scale1 = np.asarray(gamma1) / np.sqrt(np.asarray(var1) + EPS)
    shift1 = (np.asarray(beta1) + (np.asarray(b1) - np.asarray(mean1)) * scale1).astype(np.float32)
    W1p = (np.asarray(W1) * scale1[None, :]).astype(np.float32)
    scale2 = np.asarray(gamma2) / np.sqrt(np.asarray(var2) + EPS)
    shift2 = (np.asarray(beta2) + (np.asarray(b2) - np.asarray(mean2)) * scale2).astype(np.float32)
    W2p = (np.asarray(W2) * scale2[None, :]).astype(np.float32)

    xp = np.zeros((NP, F1), np.float32)
    xp[: ht["N"]] = x
    iota = np.tile(np.arange(P, dtype=np.float32)[None, :], (1, C_max))  # [1, CW]
    iota = np.broadcast_to(iota, (P, CW)).copy()
    ident = np.eye(P, dtype=np.float32)
    sh1_rep = np.broadcast_to(shift1[None, :], (P, F1)).copy()
    sh2_rep = np.broadcast_to(shift2[None, :], (P, F2)).copy()

    in_maps = []
    for c in range(n_cores):
        in_maps.append({
            "x": np.ascontiguousarray(xp[c * Nc:(c + 1) * Nc]),
            "w1": W1p, "w2": W2p,
            "shift1": sh1_rep, "shift2": sh2_rep,
            "iota": iota, "ident": ident,
            "dinv": np.ascontiguousarray(ht["dinv"][c * Nc:(c + 1) * Nc, None]),
            "offs": ht["offs_core"][c],
            "dstf": ht["dstf_core"][c],
            "didx": ht["didx_core"][c],
        })
    dims = dict(NP=NP, NWc=NWc, C_max=C_max, F1=F1, F2=F2, DG=DG)
    return ht, dims, in_maps


def kernel(x, edge_index, W1, b1, gamma1, beta1, mean1, var1,
           W2, b2, gamma2, beta2, mean2, var2, n_cores=8, _trace=False):
    from concourse.bass_utils import run_bass_kernel_spmd

    ht, dims, in_maps = _prep_inputs(
        x, edge_index, W1, b1, gamma1, beta1, mean1, var1,
        W2, b2, gamma2, beta2, mean2, var2, n_cores)
    nc = _build_program(dims["NP"], dims["NWc"], dims["C_max"],
                        dims["F1"], dims["F2"], dims["DG"], n_cores)
    try:
        res = run_bass_kernel_spmd(nc, in_maps, list(range(n_cores)), trace=_trace)
    except ModuleNotFoundError:
        res = run_bass_kernel_spmd(nc, in_maps, list(range(n_cores)), trace=False)
    E, EC = ht["E"], ht["EC"]
    out = np.empty(E, np.float32)
    for c in range(n_cores):
        e0, e1 = c * EC, min((c + 1) * EC, E)
        out[e0:e1] = res.results[c]["out"][: e1 - e0]
    kernel._last_results = res
    return out



# revision 16
# speedup vs baseline: 7.1416x; 7.1416x over previous
"""GravityAE GNN message-passing kernel for 8 TRN2 NeuronCores (Bass/Tile).

Algorithm (GCN autoencoder, BN folded into W/shift):
  scale_k = gamma_k / sqrt(var_k + eps); shift_k = beta_k + (b_k - mean_k)*scale_k
  W1p = W1 * scale1; W2p = W2 * scale2
  dinv[n] = 1/sqrt(in_degree incl self loop)
  xs1 = dinv * (x @ W1p)                      (bf16 node table, gathered by src)
  h'  = dinv * leaky(dinv[d]*segsum_d(xs1[src]) + shift1)
  z   = leaky(dinv[d]*(segsum_d(h'[src]) @ W2p) + shift2)
  out[e] = sigmoid(z[dst,64] - ||z[src,:64] - z[dst,:64]||)

Distribution: everything is dst-window sharded (each core owns 49 contiguous
128-node windows; edges+self-loops sorted by dst).  Per window the gather of
source rows is ONE batched dma_gather of node-PAIRS (idx = src>>1, int16;
pair rows keep the 256B elem-size constraint and the int16 index range) with
an even/odd copy_predicated select.  Aggregation is an indicator matmul in
PSUM.  Three bf16 AllGathers rebuild the full node tables between stages.
Decode is dst-window local: src positions are pair-gathered from the z-pos
table, dst position+mass come from the local window via a transposed one-hot
matmul; the host un-permutes the dst-sorted edge outputs.
"""
import numpy as np

P = 128
EPS = 1e-5


# --------------------------------------------------------------------------
# host-side preprocessing
# --------------------------------------------------------------------------
def _build_host_tables(N, E, src, dst, n_cores):
    import ml_dtypes

    NW = ((N + P - 1) // P + n_cores - 1) // n_cores * n_cores
    NP_ = NW * P
    NWc = NW // n_cores
    s_all = np.concatenate([src, np.arange(N, dtype=np.int64)])
    d_all = np.concatenate([dst, np.arange(N, dtype=np.int64)])
    M = E + N

    deg = np.bincount(d_all, minlength=NP_).astype(np.float64)
    dinv = np.zeros(NP_, np.float32)
    nz = deg > 0
    dinv[nz] = (1.0 / np.sqrt(deg[nz])).astype(np.float32)

    order = np.argsort(d_all, kind="stable")
    s_sorted = s_all[order]
    d_sorted = d_all[order]
    win = (d_sorted // P).astype(np.int64)
    counts = np.bincount(win, minlength=NW)
    C_max = max(1, int(np.ceil(counts.max() / P)))
    CW = C_max * P
    starts = np.zeros(NW + 1, np.int64)
    np.cumsum(counts, out=starts[1:])

    k = np.arange(M) - starts[win]            # slot within window
    pairidx = np.zeros((NW, CW), np.int16)
    oddf = np.zeros((NW, CW), np.uint8)
    dstf = np.full((NW, CW), -1.0, ml_dtypes.bfloat16)
    pairidx[win, k] = (s_sorted >> 1).astype(np.int16)
    oddf[win, k] = (s_sorted & 1).astype(np.uint8)
    dstf[win, k] = (d_sorted - win * P).astype(ml_dtypes.bfloat16)

    # per-core tables
    KW = CW // 16
    idx16 = np.empty((n_cores, 128, NWc * KW), np.int16)
    oddf_c = np.empty((n_cores, P, NWc * C_max), np.uint8)
    dstf_c = np.empty((n_cores, P, NWc * C_max), ml_dtypes.bfloat16)
    dstfT_c = np.empty((n_cores, NWc, CW), ml_dtypes.bfloat16)
    for c in range(n_cores):
        blk = pairidx[c * NWc:(c + 1) * NWc]          # [NWc, CW]
        w16 = blk.reshape(NWc, KW, 16).transpose(2, 0, 1).reshape(16, NWc * KW)
        idx16[c] = np.tile(w16, (8, 1))
        ob = oddf[c * NWc:(c + 1) * NWc].reshape(NWc, C_max, P)
        oddf_c[c] = ob.transpose(2, 0, 1).reshape(P, NWc * C_max)
        db = dstf[c * NWc:(c + 1) * NWc].reshape(NWc, C_max, P)
        dstf_c[c] = db.transpose(2, 0, 1).reshape(P, NWc * C_max)
        dstfT_c[c] = dstf[c * NWc:(c + 1) * NWc]

    # output unpermute: sorted entry j -> (core, row, col); keep real edges
    core_of = win // NWc
    wl = win % NWc
    col = wl * C_max + (k // P)
    row = k % P
    orig = order
    real = orig < E
    out_map = (orig[real], core_of[real], row[real], col[real])

    return dict(N=N, E=E, NW=NW, NP=NP_, NWc=NWc, C_max=C_max, CW=CW,
                dinv=dinv, idx16=idx16, oddf_c=oddf_c, dstf_c=dstf_c,
                dstfT_c=dstfT_c, out_map=out_map)


# --------------------------------------------------------------------------
# bass program
# --------------------------------------------------------------------------
def _build_program(NP_, NWc, C_max, F1, F2, n_cores, sim_safe=False, stages="ABCD"):
    import concourse.bass as bass
    import concourse.tile as tile
    from concourse import bacc, mybir

    dt = mybir.dt
    f32 = dt.float32
    bf16 = dt.bfloat16
    Nc = NWc * P
    CW = C_max * P
    KW = CW // 16
    Fp = F2 - 1                                 # position dims (64)
    af = mybir.ActivationFunctionType
    op = mybir.AluOpType

    nc = bacc.Bacc("TRN2", target_bir_lowering=False, debug=False,
                   num_devices=n_cores)
    xT_in = nc.declare_dram_parameter("xT", [P, Nc], bf16, isOutput=False)
    w1_in = nc.declare_dram_parameter("w1", [F1, F1], bf16, isOutput=False)
    w2_in = nc.declare_dram_parameter("w2", [F1, F2], bf16, isOutput=False)
    sh1_in = nc.declare_dram_parameter("shift1", [P, F1], f32, isOutput=False)
    sh2_in = nc.declare_dram_parameter("shift2", [P, F2], f32, isOutput=False)
    dinv_in = nc.declare_dram_parameter("dinv", [P, NWc], f32, isOutput=False)
    idx_in = nc.declare_dram_parameter("idx16", [P, NWc * KW], dt.int16, isOutput=False)
    odd_in = nc.declare_dram_parameter("oddf", [P, NWc * C_max], dt.uint8, isOutput=False)
    dstf_in = nc.declare_dram_parameter("dstf", [P, NWc * C_max], bf16, isOutput=False)
    dstfT_in = nc.declare_dram_parameter("dstfT", [NWc, CW], bf16, isOutput=False)
    out_dram = nc.declare_dram_parameter("out", [P, NWc * C_max], f32, isOutput=True)

    rg = [list(range(n_cores))]

    with tile.TileContext(nc) as tc:
        with (
            tc.tile_pool(name="const", bufs=1) as cpool,
            tc.tile_pool(name="sbuf", bufs=2) as pool,
            tc.tile_pool(name="psA", bufs=2, space="PSUM") as psA,
            tc.tile_pool(name="psZ", bufs=1, space="PSUM") as psZ,
            tc.tile_pool(name="psD", bufs=1, space="PSUM") as psD,
            tc.tile_pool(name="dram", bufs=1, space="DRAM") as dpool,
        ):
            # ---- constants ----
            w1_t = cpool.tile([F1, F1], bf16)
            w2_t = cpool.tile([F1, F2], bf16)
            sh1_t = cpool.tile([P, F1], f32)
            sh2_t = cpool.tile([P, F2], f32)
            xT_t = cpool.tile([P, Nc], bf16)
            dinv_t = cpool.tile([P, NWc], f32)
            idx_t = cpool.tile([P, NWc * KW], dt.int16)
            odd_t = cpool.tile([P, NWc * C_max], dt.uint8)
            dstf_t = cpool.tile([P, NWc * C_max], bf16)
            nc.sync.dma_start(out=w1_t[:], in_=w1_in[:])
            nc.sync.dma_start(out=w2_t[:], in_=w2_in[:])
            nc.sync.dma_start(out=sh1_t[:], in_=sh1_in[:])
            nc.sync.dma_start(out=sh2_t[:], in_=sh2_in[:])
            nc.sync.dma_start(out=xT_t[:], in_=xT_in[:])
            nc.sync.dma_start(out=dinv_t[:], in_=dinv_in[:])
            nc.sync.dma_start(out=idx_t[:], in_=idx_in[:])
            nc.sync.dma_start(out=odd_t[:], in_=odd_in[:])
            nc.sync.dma_start(out=dstf_t[:], in_=dstf_in[:])

            # device-built iotas
            iota_i = cpool.tile([P, C_max, P], dt.int16)
            nc.gpsimd.iota(iota_i[:], pattern=[[0, C_max], [1, P]], base=0,
                           channel_multiplier=0)
            iota_t = cpool.tile([P, C_max, P], bf16)
            nc.vector.tensor_copy(iota_t[:], iota_i[:])
            iotaP_i = cpool.tile([P, 1], dt.int32)
            nc.gpsimd.iota(iotaP_i[:], pattern=[[0, 1]], base=0,
                           channel_multiplier=1)
            iotaP_t = cpool.tile([P, 1], bf16)
            nc.vector.tensor_copy(iotaP_t[:], iotaP_i[:])

            posmj_t = cpool.tile([P, NWc * P], bf16)
            nc.vector.memset(posmj_t[:], 0.0)
            stage_d2 = cpool.tile([P, NWc * C_max], f32)
            stage_mj = cpool.tile([P, NWc * C_max], f32)
            nc.vector.memset(stage_d2[:], 0.0)
            nc.vector.memset(stage_mj[:], 0.0)

            # ---- collective buffers (pair-packed views) ----
            ag1 = dpool.tile([Nc, F1], bf16)
            xs1_full = dpool.tile([NP_ // 2, 2 * F1], bf16, addr_space="Shared")
            ag2 = dpool.tile([Nc, F1], bf16)
            h_full = dpool.tile([NP_ // 2, 2 * F1], bf16, addr_space="Shared")
            ag3 = dpool.tile([Nc, Fp], bf16)
            pos_full = dpool.tile([NP_ // 2, 2 * Fp], bf16, addr_space="Shared")

            # ---- stage A: xs1 shard = dinv * (x @ W1p) ----
            for w in range(NWc):
                ps = psA.tile([P, F1], f32, tag="mm")
                nc.tensor.matmul(ps[:], xT_t[:, w * P:(w + 1) * P], w1_t[:],
                                 start=True, stop=True)
                xs = pool.tile([P, F1], bf16, tag="axs")
                nc.vector.tensor_scalar_mul(xs[:], ps[:], dinv_t[:, w:w + 1])
                nc.sync.dma_start(out=ag1[w * P:(w + 1) * P, :], in_=xs[:])

            nc.gpsimd.collective_compute(
                "AllGather", mybir.AluOpType.bypass,
                ins=[ag1.opt()], outs=[xs1_full.opt()], replica_groups=rg)

            # ---- shared gather+select+S helper ----
            GMAX = 8                     # chunks per dma_gather (Q7 ~1024-idx cap)

            def gather_select(full_tab, elem, half, w, tagp):
                msg = pool.tile([P, C_max, elem], bf16, tag=tagp + "msg")
                for g0 in range(0, C_max, GMAX):
                    g1 = min(g0 + GMAX, C_max)
                    ni = (g1 - g0) * P
                    nc.gpsimd.dma_gather(
                        out_ap=msg[:, g0:g1, :], in_ap=full_tab[:],
                        idxs_ap=idx_t[:, w * KW + g0 * 8:w * KW + g1 * 8],
                        num_idxs=ni, num_idxs_reg=ni, elem_size=elem)
                nc.vector.copy_predicated(
                    msg[:, :, 0:half],
                    odd_t[:, w * C_max:(w + 1) * C_max]
                    .rearrange("p (c o) -> p c o", o=1)
                    .to_broadcast([P, C_max, half]),
                    msg[:, :, half:2 * half])
                return msg

            def build_S(w, tagp):
                S = pool.tile([P, C_max, P], bf16, tag="S")
                nc.vector.tensor_tensor(
                    out=S[:],
                    in0=dstf_t[:, w * C_max:(w + 1) * C_max]
                    .rearrange("p (c o) -> p c o", o=1)
                    .to_broadcast([P, C_max, P]),
                    in1=iota_t[:], op=op.is_equal)
                return S

            # ---- stage B: h' windows ----
            for w in range(NWc if "B" in stages else 0):
                msg = gather_select(xs1_full, 2 * F1, F1, w, "b")
                S = build_S(w, "b")
                ps = psA.tile([P, F1], f32, tag="mm")
                for c in range(C_max):
                    nc.tensor.matmul(ps[:], S[:, c, :], msg[:, c, 0:F1],
                                     start=(c == 0), stop=(c == C_max - 1))
                t = pool.tile([P, F1], f32, tag="bt")
                nc.vector.scalar_tensor_tensor(
                    out=t[:], in0=ps[:], scalar=dinv_t[:, w:w + 1], in1=sh1_t[:],
                    op0=op.mult, op1=op.add)
                h = pool.tile([P, F1], f32, tag="bh")
                u = pool.tile([P, F1], f32, tag="bu")
                nc.scalar.activation(u[:], t[:], af.Copy, scale=0.1)
                nc.vector.tensor_tensor(out=h[:], in0=t[:], in1=u[:], op=op.max)
                hp = pool.tile([P, F1], bf16, tag="bhp")
                nc.vector.tensor_scalar_mul(hp[:], h[:], dinv_t[:, w:w + 1])
                nc.sync.dma_start(out=ag2[w * P:(w + 1) * P, :], in_=hp[:])

            nc.gpsimd.collective_compute(
                "AllGather", mybir.AluOpType.bypass,
                ins=[ag2.opt()], outs=[h_full.opt()], replica_groups=rg)

            # ---- stage C: z windows (aggregate then transform) ----
            for w in range(NWc if "C" in stages else 0):
                msg = gather_select(h_full, 2 * F1, F1, w, "c")
                S = build_S(w, "c")
                psT = psA.tile([P, F1], f32, tag="mm")
                for c in range(C_max):
                    nc.tensor.matmul(psT[:], msg[:, c, 0:F1], S[:, c, :],
                                     start=(c == 0), stop=(c == C_max - 1))
                aggb = pool.tile([P, F1], bf16, tag="cagg")
                nc.vector.tensor_copy(aggb[:], psT[:])
                psz = psZ.tile([P, F2], f32, tag="z")
                nc.tensor.matmul(psz[:], aggb[:], w2_t[:], start=True, stop=True)
                tz = pool.tile([P, F2], f32, tag="ct")
                nc.vector.scalar_tensor_tensor(
                    out=tz[:], in0=psz[:], scalar=dinv_t[:, w:w + 1], in1=sh2_t[:],
                    op0=op.mult, op1=op.add)
                z = pool.tile([P, F2], f32, tag="cz")
                uz = pool.tile([P, F2], f32, tag="cuz")
                nc.scalar.activation(uz[:], tz[:], af.Copy, scale=0.1)
                nc.vector.tensor_tensor(out=z[:], in0=tz[:], in1=uz[:], op=op.max)
                nc.vector.tensor_copy(posmj_t[:, w * P:w * P + F2], z[:])
                nc.sync.dma_start(out=ag3[w * P:(w + 1) * P, :],
                                  in_=posmj_t[:, w * P:w * P + Fp])

            nc.gpsimd.collective_compute(
                "AllGather", mybir.AluOpType.bypass,
                ins=[ag3.opt()], outs=[pos_full.opt()], replica_groups=rg)

            # ---- stage D: decode (dst-window local) ----
            for w in range(NWc if "D" in stages else 0):
                msg = gather_select(pos_full, 2 * Fp, Fp, w, "d")
                dstrep = pool.tile([P, CW], bf16, tag="drep")
                nc.sync.dma_start(out=dstrep[:],
                                  in_=dstfT_in[w:w + 1, :].to_broadcast([P, CW]))
                ST = pool.tile([P, CW], bf16, tag="dST")
                nc.vector.tensor_tensor(out=ST[:], in0=dstrep[:],
                                        in1=iotaP_t[:].to_broadcast([P, CW]),
                                        op=op.is_equal)
                psd = psD.tile([P, C_max, P], f32, tag="dall")
                for c in range(C_max):
                    nc.tensor.matmul(psd[:, c, :], ST[:, c * P:(c + 1) * P],
                                     posmj_t[:, w * P:(w + 1) * P],
                                     start=True, stop=True)
                diff = pool.tile([P, C_max, Fp], bf16, tag="ddiff")
                nc.vector.tensor_tensor(out=diff[:], in0=msg[:, :, 0:Fp],
                                        in1=psd[:, :, 0:Fp], op=op.subtract)
                sq = pool.tile([P, C_max, Fp], f32, tag="dsq")
                nc.vector.tensor_tensor(out=sq[:], in0=diff[:], in1=diff[:],
                                        op=op.mult)
                nc.vector.reduce_sum(
                    out=stage_d2[:, w * C_max:(w + 1) * C_max]
                    .rearrange("p (c o) -> p c o", o=1),
                    in_=sq[:], axis=mybir.AxisListType.X)
                nc.vector.tensor_copy(stage_mj[:, w * C_max:(w + 1) * C_max],
                                      psd[:, :, Fp])

            # ---- finale: sigmoid(mj - sqrt(d2)) ----
            sd = cpool.tile([P, NWc * C_max], f32)
            nc.scalar.activation(sd[:], stage_d2[:], af.Sqrt)
            sv = cpool.tile([P, NWc * C_max], f32)
            nc.vector.tensor_tensor(out=sv[:], in0=stage_mj[:], in1=sd[:],
                                    op=op.subtract)
            so = cpool.tile([P, NWc * C_max], f32)
            nc.scalar.activation(so[:], sv[:], af.Sigmoid)
            nc.sync.dma_start(out=out_dram[:], in_=so[:])
    nc.compile()
    return nc


_PROG_CACHE = {}
_SIM_SAFE = False


def _get_program(NP_, NWc, C_max, F1, F2, n_cores):
    key = (NP_, NWc, C_max, F1, F2, n_cores, _SIM_SAFE)
    if key not in _PROG_CACHE:
        _PROG_CACHE[key] = _build_program(NP_, NWc, C_max, F1, F2, n_cores,
                                          sim_safe=_SIM_SAFE)
    return _PROG_CACHE[key]


# --------------------------------------------------------------------------
# public entry
# --------------------------------------------------------------------------
def kernel(x, edge_index, W1, b1, gamma1, beta1, mean1, var1,
           W2, b2, gamma2, beta2, mean2, var2, n_cores=8, _trace=False):
    import ml_dtypes
    from concourse.bass_utils import run_bass_kernel_spmd

    x = np.asarray(x, np.float32)
    edge_index = np.asarray(edge_index)
    N, F1 = x.shape
    E = edge_index.shape[1]
    F2 = np.asarray(W2).shape[1]
    src = edge_index[0].astype(np.int64)
    dst = edge_index[1].astype(np.int64)

    ht = _build_host_tables(N, E, src, dst, n_cores)
    NP_, NWc, C_max = ht["NP"], ht["NWc"], ht["C_max"]
    Nc = NWc * P

    scale1 = np.asarray(gamma1) / np.sqrt(np.asarray(var1) + EPS)
    shift1 = (np.asarray(beta1) + (np.asarray(b1) - np.asarray(mean1)) * scale1).astype(np.float32)
    W1p = (np.asarray(W1) * scale1[None, :]).astype(ml_dtypes.bfloat16)
    scale2 = np.asarray(gamma2) / np.sqrt(np.asarray(var2) + EPS)
    shift2 = (np.asarray(beta2) + (np.asarray(b2) - np.asarray(mean2)) * scale2).astype(np.float32)
    W2p = (np.asarray(W2) * scale2[None, :]).astype(ml_dtypes.bfloat16)

    xp = np.zeros((NP_, F1), np.float32)
    xp[:N] = x
    sh1_rep = np.broadcast_to(shift1[None, :], (P, F1)).copy()
    sh2_rep = np.broadcast_to(shift2[None, :], (P, F2)).copy()

    in_maps = []
    for c in range(n_cores):
        xc = xp[c * Nc:(c + 1) * Nc]
        in_maps.append({
            "xT": np.ascontiguousarray(xc.T).astype(ml_dtypes.bfloat16),
            "w1": W1p, "w2": W2p,
            "shift1": sh1_rep, "shift2": sh2_rep,
            "dinv": np.ascontiguousarray(
                ht["dinv"][c * Nc:(c + 1) * Nc].reshape(NWc, P).T),
            "idx16": ht["idx16"][c],
            "oddf": ht["oddf_c"][c],
            "dstf": ht["dstf_c"][c],
            "dstfT": ht["dstfT_c"][c],
        })

    nc = _get_program(NP_, NWc, C_max, F1, F2, n_cores)
    try:
        res = run_bass_kernel_spmd(nc, in_maps, list(range(n_cores)), trace=_trace)
    except ModuleNotFoundError:
        res = run_bass_kernel_spmd(nc, in_maps, list(range(n_cores)), trace=False)

    orig, core_of, row, col = ht["out_map"]
    per_core = np.stack([res.results[c]["out"] for c in range(n_cores)])
    out = np.empty(E, np.float32)
    out[orig] = per_core[core_of, row, col]
    kernel._last_results = res
    return out
